# revision 20
# baseline (speedup 1.0000x reference)
"""nn_BinaryMoSLinear Trainium2 kernel: 8 NeuronCores, data-parallel over tokens.

kernel(**inputs) takes the FULL reference.setup_inputs() tensors and returns
the FULL [4, 2048, 4096] f32 output. Tokens are sharded 1024/core; weight,
bias, gate and channel scales are replicated (no collectives).

v3: all matmul operands bf16 (sign(weight) exact in bf16, host pre-binarized;
x_s rounded to bf16, ~5e-3 rel err vs the 2e-2 gate). Token-halves pipeline:
half A (512 tokens) runs router -> softmax -> in_scale, then its main matmuls
start while half B's router/softmax/in_scale execute interleaved on PE/DVE/
GpSimd inside half A's windows. x_s is computed in place over the x tiles
(x streamed once). PSUM budget: 1 logits + 3 scratch + 4 main banks; main
windows are (oc, token-pair) = 2 banks with a 4-ring so bank reuse is always
one full window behind the drain. W and ocs are streamed once per half.
"""
from contextlib import ExitStack

import concourse.bass as bass
import concourse.mybir as mybir

F32 = mybir.dt.float32
BF16 = mybir.dt.bfloat16
FP8 = mybir.dt.float8e4
AF = mybir.ActivationFunctionType
OP = mybir.AluOpType


def build_moe8(ctx, tc, outs, ins, cfg):
    nc = tc.nc
    H, O, Nc, E = cfg["H"], cfg["O"], cfg["Nc"], cfg["E"]
    ow = cfg["ow"]
    NH = H // 128            # 32 h-chunks
    OC = O // ow             # 8 output column blocks
    y = outs["y"]

    pool = ctx.enter_context(tc.tile_pool(name="sb", bufs=1))
    ctx.enter_context(nc.allow_low_precision(reason="bf16 pipeline, ~5e-3"))
    psum = ctx.enter_context(tc.tile_pool(name="ps", bufs=1, space="PSUM"))

    # ---- constants (tiles declared here, DMAs woven into the x stream) ----
    gwp = pool.tile([128, NH * E], BF16, name="gwp", tag="gwp", bufs=1)
    ics_t = pool.tile([E, H], BF16, name="ics", tag="ics", bufs=1)
    ones_e1 = pool.tile([E, 1], BF16, name="ones_e1", tag="ones_e1", bufs=1)
    ones_1e = pool.tile([1, E], BF16, name="ones_1e", tag="ones_1e", bufs=1)
    eye_e = pool.tile([E, E], BF16, name="eye_e", tag="eye_e", bufs=1)

    def dma_consts(h):
        if h == 0:
            nc.scalar.dma_start(gwp[:], ins["gwP"][:, :])
        elif h == 2:
            nc.scalar.dma_start(ics_t[:], ins["ics"][:, :])
        elif h == 4:
            nc.sync.dma_start(ones_e1[:], ins["ones_e"][:, 0:1])
            nc.sync.dma_start(ones_1e[:], ins["ones_e"][0:1, :])
            nc.scalar.dma_start(eye_e[:], ins["eye_e"][:, :])

    xh = {}      # (half, h) -> [128, 512] bf16 x tile, becomes x_s in place
    rj = {}      # tc -> [128, E] bf16 routing weights, token-major
    rts = {}     # half -> [E, 512] bf16 routing weights
    wst = {}     # (gen, oc, h) -> [128, ow] bf16 signed weight slab
    logits = {}
    bias_t = {}
    ocsb_t = {}
    os_t = {}
    mains = {}

    def dma_x(hf, h):
        t = pool.tile([128, 512], BF16, name=f"x{hf}_{h}", tag=f"xh{hf}",
                      bufs=NH)
        eng = nc.sync if h % 2 == 0 else nc.scalar
        eng.dma_start(t[:], ins["xT"][h * 128:(h + 1) * 128,
                                      hf * 512:(hf + 1) * 512])
        xh[(hf, h)] = t

    def dma_w(gen, oc, h):
        t = pool.tile([128, ow], FP8, name=f"w{gen}_{oc}_{h}", tag="wst",
                      bufs=48)
        eng = nc.sync if h % 2 == 0 else nc.scalar
        eng.dma_start(t[:], ins["wbT"][h * 128:(h + 1) * 128,
                                       oc * ow:(oc + 1) * ow])
        wst[(gen, oc, h)] = t

    def dma_oc_consts(gen, oc):
        b = pool.tile([128, ow], F32, name=f"bias{gen}_{oc}", tag="bias",
                      bufs=2)
        nc.scalar.dma_start(b[:], ins["bias2"][:, oc * ow:(oc + 1) * ow])
        bias_t[(gen, oc)] = b
        for e in range(E):
            t = pool.tile([128, ow], F32, name=f"ocsb{gen}_{oc}_{e}",
                          tag="ocsb", bufs=2 * E)
            nc.scalar.dma_start(t[:], ins["ocsb"][e * 128:(e + 1) * 128,
                                                  oc * ow:(oc + 1) * ow])
            ocsb_t[(gen, oc, e)] = t

    # ---- per-half router pieces ----
    def router_mm(hf, h):
        if hf not in logits:
            logits[hf] = psum.tile([E, 512], F32, name=f"logits{hf}",
                                   tag="plog", bufs=1)
        nc.tensor.matmul(logits[hf][:], gwp[:, h * E:(h + 1) * E],
                         xh[(hf, h)][:], start=(h == 0), stop=(h == NH - 1))

    exs = {}
    rcps = {}

    def softmax_a(hf):
        ex = pool.tile([E, 512], BF16, name=f"ex{hf}", tag="ex", bufs=2)
        nc.scalar.activation(ex[:], logits.pop(hf)[:], AF.Exp)
        exs[hf] = ex
        ssum = psum.tile([1, 512], F32, name=f"ssum{hf}", tag="psm", bufs=3)
        nc.tensor.matmul(ssum[:], ones_e1[:], ex[:], start=True, stop=True)
        rcp32 = pool.tile([1, 512], F32, name=f"rcp32{hf}", tag="rcp32",
                          bufs=2)
        nc.vector.reciprocal_approx_fast(rcp32[:], ssum[:])
        rcpb = pool.tile([1, 512], BF16, name=f"rcpb{hf}", tag="rcpb", bufs=2)
        nc.vector.tensor_copy(rcpb[:], rcp32[:])
        rcps[hf] = rcpb

    def softmax_b(hf):
        bc = psum.tile([E, 512], F32, name=f"bc{hf}", tag="psm", bufs=3)
        nc.tensor.matmul(bc[:], ones_1e[:], rcps.pop(hf)[:], start=True,
                         stop=True)
        rt = pool.tile([E, 512], BF16, name=f"rt{hf}", tag="rt", bufs=2)
        nc.vector.tensor_tensor(rt[:], exs.pop(hf)[:], bc[:], OP.mult)
        rts[hf] = rt
        for j in range(4):
            rtp = psum.tile([128, E], BF16, name=f"rtp{hf}_{j}", tag="psm",
                            bufs=3)
            nc.tensor.transpose(rtp[:], rt[:, j * 128:(j + 1) * 128], eye_e[:])
            r = pool.tile([128, E], F32, name=f"rj{hf}_{j}",
                          tag=f"rj{hf}_{j}", bufs=1)
            nc.vector.tensor_copy(r[:], rtp[:])
            rj[hf * 4 + j] = r

    def in_scale(hf, h):
        # x_s = x * (r @ ics), written in place over the x tile
        isp = psum.tile([128, 512], F32, name=f"isp{hf}_{h}", tag="psm",
                        bufs=3)
        nc.tensor.matmul(isp[:], ics_t[:, h * 128:(h + 1) * 128],
                         rts[hf][:], start=True, stop=True)
        xt = xh[(hf, h)]
        if h % 3 != 2:
            nc.vector.tensor_tensor(xt[:], xt[:], isp[:], OP.mult)
        else:
            # gpsimd TT is ~2x slower than DVE; give it 1/3 of the tiles
            icp = pool.tile([128, 512], F32, name=f"icp{hf}_{h}", tag="icp",
                            bufs=3)
            nc.scalar.activation(icp[:], isp[:], AF.Copy)
            nc.gpsimd.tensor_tensor(xt[:], xt[:], icp[:], OP.mult)

    # ---- main-phase window pieces ----
    def emit_os(gen, oc, tcs):
        for tcc in tcs:
            t = pool.tile([128, ow], F32, name=f"os{gen}_{oc}_{tcc}",
                          tag="os", bufs=8)
            r = rj[tcc]
            nc.vector.tensor_scalar_mul(t[:], ocsb_t[(gen, oc, 0)][:],
                                        r[:, 0:1])
            for e in range(1, E):
                nc.vector.scalar_tensor_tensor(t[:], ocsb_t[(gen, oc, e)][:],
                                               r[:, e:e + 1], t[:],
                                               OP.mult, OP.add)
            os_t[(oc, tcc)] = t

    def emit_drain(gen, oc, tcc):
        yt = pool.tile([128, ow], F32, name=f"yt{gen}_{oc}_{tcc}", tag="yt",
                       bufs=6)
        nc.vector.tensor_tensor(yt[:], mains.pop((oc, tcc))[:],
                                os_t.pop((oc, tcc))[:], OP.mult)
        yt2 = pool.tile([128, ow], F32, name=f"yt2{gen}_{oc}_{tcc}",
                        tag="yt2", bufs=6)
        nc.gpsimd.tensor_tensor(yt2[:], yt[:], bias_t[(gen, oc)][:], OP.add)
        nc.scalar.dma_start(y[tcc * 128:(tcc + 1) * 128,
                              oc * ow:(oc + 1) * ow], yt2[:])

    # ---- prologue: half A router + softmax; in_scale A fused into w0 ----
    for h in range(NH):
        dma_x(0, h)
        dma_consts(h)
        router_mm(0, h)
    softmax_a(0)
    softmax_b(0)
    dma_oc_consts(0, 0)
    for h in range(NH):
        dma_w(0, 0, h)
    in_scale(0, 0)
    in_scale(0, 1)

    # ---- main: windows (gen, oc, pair); gen 0 = half A (tc 0-3),
    #      gen 1 = half B (tc 4-7). Window = 2 PSUM banks, ring of 4. ----
    windows = [(g, oc, p) for g in range(2) for oc in range(OC)
               for p in range(2)]

    # extra ops (half-B prologue) interleaved into half-A windows, per h step
    extras = {}                      # (widx, h) -> list of callables
    for h in range(NH):              # window 0: stream x half B,
        extras.setdefault((0, h), []).append(lambda h=h: dma_x(1, h))
    for h in range(NH - 2):          # ... and in_scale A two steps ahead
        extras.setdefault((0, h), []).append(lambda h=h: in_scale(0, h + 2))
    for h2 in range(NH):             # windows 1-2: router B (1 mm / step)
        w = 1 + h2 // 16
        extras.setdefault((w, (h2 * 2) % 32), []).append(
            lambda h2=h2: router_mm(1, h2))
    extras.setdefault((3, 0), []).append(lambda: softmax_a(1))
    extras.setdefault((3, 16), []).append(lambda: softmax_b(1))
    for h2 in range(NH):             # windows 4-14: in_scale B (3 / window)
        w = 4 + h2 // 3
        extras.setdefault((w, (h2 % 3) * 10 + 5), []).append(
            lambda h2=h2: in_scale(1, h2))

    prev = None
    for widx, (gen, oc, p) in enumerate(windows):
        tcs = [gen * 4 + p * 2, gen * 4 + p * 2 + 1]
        last = widx == len(windows) - 1
        if prev is not None:
            for tcc in prev[1]:
                emit_drain(prev[0], prev[2], tcc)
        emit_os(gen, oc, tcs)
        for tcc in tcs:
            mains[(oc, tcc)] = psum.tile([128, ow], F32,
                                         name=f"mp{gen}_{oc}_{tcc}",
                                         tag="mps", bufs=4)
        if p == 1:
            ngen, noc = (gen, oc + 1) if oc + 1 < OC else (gen + 1, 0)
            if ngen < 2:
                dma_oc_consts(ngen, noc)
        if last:
            # column-major: finish tc by tc so the tail drain is short
            for tcc in tcs:
                for h in range(NH):
                    w = wst[(gen, oc, h)] if tcc == tcs[0] else \
                        wst.pop((gen, oc, h))
                    nc.tensor.matmul(mains[(oc, tcc)][:],
                                     xh[(gen, h)][:, (tcc % 4) * 128:
                                                   (tcc % 4 + 1) * 128],
                                     w[:], start=(h == 0), stop=(h == NH - 1))
                if tcc != tcs[1]:
                    emit_drain(gen, oc, tcc)
            # final tc drained in halves so DVE/GpSimd/DMA pipeline
            tcc = tcs[1]
            mp = mains.pop((oc, tcc))
            ot = os_t.pop((oc, tcc))
            for k in range(2):
                sl = slice(k * (ow // 2), (k + 1) * (ow // 2))
                yt = pool.tile([128, ow // 2], F32, name=f"ytf{k}", tag="ytf",
                               bufs=2)
                nc.vector.tensor_tensor(yt[:], mp[:, sl], ot[:, sl], OP.mult)
                yt2 = pool.tile([128, ow // 2], F32, name=f"ytf2{k}",
                                tag="ytf2", bufs=2)
                nc.vector.tensor_tensor(yt2[:], yt[:],
                                        bias_t[(gen, oc)][:, sl], OP.add)
                nc.scalar.dma_start(y[tcc * 128:(tcc + 1) * 128,
                                      oc * ow + k * (ow // 2):
                                      oc * ow + (k + 1) * (ow // 2)], yt2[:])
            prev = None
            break
        for h in range(NH):
            for fn in extras.pop((widx, h), ()):
                fn()
            if p == 1:
                ngen, noc = (gen, oc + 1) if oc + 1 < OC else (gen + 1, 0)
                if ngen < 2:
                    dma_w(ngen, noc, h)
            w = wst[(gen, oc, h)] if p == 0 else wst.pop((gen, oc, h))
            for tcc in tcs:
                nc.tensor.matmul(mains[(oc, tcc)][:],
                                 xh[(gen, h)][:, (tcc % 4) * 128:
                                               (tcc % 4 + 1) * 128],
                                 w[:], start=(h == 0), stop=(h == NH - 1))
        prev = (gen, tcs, oc)
    assert not extras, f"unconsumed extras: {list(extras)[:4]}"


import numpy as np
import ml_dtypes

NCORES = 8
B, S, H, O, E = 4, 2048, 4096, 4096, 4
N = B * S
Nc = N // NCORES
CFG = dict(H=H, O=O, Nc=Nc, E=E, ow=512)
BF16_NP = ml_dtypes.bfloat16
FP8_NP = ml_dtypes.float8_e4m3

TRACE = False
LAST_EXEC_NS = None
LAST_TRACE_PATH = None
_NC_CACHE = None


def _get_nc():
    global _NC_CACHE
    if _NC_CACHE is None:
        import concourse.bacc as bacc
        import concourse.tile as tile
        nc = bacc.Bacc("TRN2", target_bir_lowering=False, debug=False,
                       num_devices=NCORES)
        ins_aps = {
            "xT": nc.dram_tensor("xT", [H, Nc], BF16, kind="ExternalInput").ap(),
            "wbT": nc.dram_tensor("wbT", [H, O], FP8, kind="ExternalInput").ap(),
            "gwP": nc.dram_tensor("gwP", [128, (H // 128) * E], BF16,
                                  kind="ExternalInput").ap(),
            "ics": nc.dram_tensor("ics", [E, H], BF16, kind="ExternalInput").ap(),
            "ocsb": nc.dram_tensor("ocsb", [E * 128, O], F32,
                                   kind="ExternalInput").ap(),
            "bias2": nc.dram_tensor("bias2", [128, O], F32,
                                    kind="ExternalInput").ap(),
            "ones_e": nc.dram_tensor("ones_e", [E, E], BF16,
                                     kind="ExternalInput").ap(),
            "eye_e": nc.dram_tensor("eye_e", [E, E], BF16,
                                    kind="ExternalInput").ap(),
        }
        outs_aps = {"y": nc.dram_tensor("y", [Nc, O], F32,
                                        kind="ExternalOutput").ap()}
        with tile.TileContext(nc) as tc:
            with ExitStack() as ctx:
                build_moe8(ctx, tc, outs_aps, ins_aps, CFG)
        nc.compile()
        _NC_CACHE = nc
    return _NC_CACHE


def kernel(x, weight, bias, gate_w, in_channel_scale, out_channel_scale):
    """Full inputs in, full output out; distributes over 8 NeuronCores."""
    global LAST_EXEC_NS, LAST_TRACE_PATH
    from concourse.bass_utils import run_bass_kernel_spmd

    x = np.asarray(x, dtype=np.float32)
    weight = np.asarray(weight, dtype=np.float32)
    bias = np.asarray(bias, dtype=np.float32)
    gate_w = np.asarray(gate_w, dtype=np.float32)
    ics = np.asarray(in_channel_scale, dtype=np.float32)
    ocs = np.asarray(out_channel_scale, dtype=np.float32)

    nc = _get_nc()
    xf = x.reshape(N, H)
    wbT = np.sign(weight).T.astype(FP8_NP)
    gwP = np.ascontiguousarray(
        gate_w.T.reshape(H // 128, 128, E).transpose(1, 0, 2)
        .reshape(128, (H // 128) * E)).astype(BF16_NP)
    ics_b = ics.astype(BF16_NP)
    bias2 = np.ascontiguousarray(np.broadcast_to(bias[None, :], (128, O)))
    ocsb = np.ascontiguousarray(
        np.broadcast_to(ocs[:, None, :], (E, 128, O)).reshape(E * 128, O))
    ones_e = np.ones((E, E), dtype=BF16_NP)
    eye_e = np.eye(E, dtype=BF16_NP)
    in_maps = []
    for c in range(NCORES):
        in_maps.append({
            "xT": np.ascontiguousarray(xf[c * Nc:(c + 1) * Nc, :].T).astype(BF16_NP),
            "wbT": wbT, "gwP": gwP, "ics": ics_b, "ocsb": ocsb,
            "bias2": bias2, "ones_e": ones_e, "eye_e": eye_e,
        })
    res = run_bass_kernel_spmd(nc, in_maps, core_ids=list(range(NCORES)),
                               trace=TRACE)
    if TRACE:
        LAST_EXEC_NS = res.exec_time_ns
        if res.instructions_and_trace:
            LAST_TRACE_PATH = res.instructions_and_trace[1]
    yfull = np.concatenate([res.results[c]["y"] for c in range(NCORES)], axis=0)
    return yfull.reshape(B, S, O)


# revision 21
# speedup vs baseline: 1.0195x; 1.0195x over previous
"""nn_BinaryMoSLinear Trainium2 kernel: 8 NeuronCores, data-parallel over tokens.

kernel(**inputs) takes the FULL reference.setup_inputs() tensors and returns
the FULL [4, 2048, 4096] f32 output. Tokens are sharded 1024/core; weight,
bias, gate and channel scales are replicated (no collectives).

v3: all matmul operands bf16 (sign(weight) exact in bf16, host pre-binarized;
x_s rounded to bf16, ~5e-3 rel err vs the 2e-2 gate). Token-halves pipeline:
half A (512 tokens) runs router -> softmax -> in_scale, then its main matmuls
start while half B's router/softmax/in_scale execute interleaved on PE/DVE/
GpSimd inside half A's windows. x_s is computed in place over the x tiles
(x streamed once). PSUM budget: 1 logits + 3 scratch + 4 main banks; main
windows are (oc, token-pair) = 2 banks with a 4-ring so bank reuse is always
one full window behind the drain. W and ocs are streamed once per half.
"""
from contextlib import ExitStack

import concourse.bass as bass
import concourse.mybir as mybir

F32 = mybir.dt.float32
BF16 = mybir.dt.bfloat16
FP8 = mybir.dt.float8e4
AF = mybir.ActivationFunctionType
OP = mybir.AluOpType


def build_moe8(ctx, tc, outs, ins, cfg):
    nc = tc.nc
    H, O, Nc, E = cfg["H"], cfg["O"], cfg["Nc"], cfg["E"]
    ow = cfg["ow"]
    NH = H // 128            # 32 h-chunks
    OC = O // ow             # 8 output column blocks
    y = outs["y"]

    pool = ctx.enter_context(tc.tile_pool(name="sb", bufs=1))
    ctx.enter_context(nc.allow_low_precision(reason="bf16 pipeline, ~5e-3"))
    psum = ctx.enter_context(tc.tile_pool(name="ps", bufs=1, space="PSUM"))

    # ---- constants (tiles declared here, DMAs woven into the x stream) ----
    gwp = pool.tile([128, NH * E], BF16, name="gwp", tag="gwp", bufs=1)
    ics_t = pool.tile([E, H], BF16, name="ics", tag="ics", bufs=1)
    ones_e1 = pool.tile([E, 1], BF16, name="ones_e1", tag="ones_e1", bufs=1)
    ones_1e = pool.tile([1, E], BF16, name="ones_1e", tag="ones_1e", bufs=1)
    eye_e = pool.tile([E, E], BF16, name="eye_e", tag="eye_e", bufs=1)

    def dma_consts(h):
        if h == 0:
            nc.scalar.dma_start(gwp[:], ins["gwP"][:, :])
        elif h == 2:
            nc.scalar.dma_start(ics_t[:], ins["ics"][:, :])
        elif h == 4:
            nc.sync.dma_start(ones_e1[:], ins["ones_e"][:, 0:1])
            nc.sync.dma_start(ones_1e[:], ins["ones_e"][0:1, :])
            nc.scalar.dma_start(eye_e[:], ins["eye_e"][:, :])

    xh = {}      # (half, h) -> [128, 512] bf16 x tile, becomes x_s in place
    rj = {}      # tc -> [128, E] bf16 routing weights, token-major
    rts = {}     # half -> [E, 512] bf16 routing weights
    wst = {}     # (gen, oc, h) -> [128, ow] bf16 signed weight slab
    logits = {}
    bias_t = {}
    ocsb_t = {}
    os_t = {}
    mains = {}

    def dma_x(hf, h):
        t = pool.tile([128, 512], BF16, name=f"x{hf}_{h}", tag=f"xh{hf}",
                      bufs=NH)
        eng = nc.sync if h % 2 == 0 else nc.scalar
        eng.dma_start(t[:], ins["xT"][h * 128:(h + 1) * 128,
                                      hf * 512:(hf + 1) * 512])
        xh[(hf, h)] = t

    def dma_w(gen, oc, h):
        t = pool.tile([128, ow], FP8, name=f"w{gen}_{oc}_{h}", tag="wst",
                      bufs=48)
        eng = nc.sync if h % 2 == 0 else nc.scalar
        eng.dma_start(t[:], ins["wbT"][h * 128:(h + 1) * 128,
                                       oc * ow:(oc + 1) * ow])
        wst[(gen, oc, h)] = t

    def dma_oc_consts(gen, oc):
        b = pool.tile([128, ow], F32, name=f"bias{gen}_{oc}", tag="bias",
                      bufs=2)
        nc.scalar.dma_start(b[:], ins["bias2"][:, oc * ow:(oc + 1) * ow])
        bias_t[(gen, oc)] = b
        for e in range(E):
            t = pool.tile([128, ow], F32, name=f"ocsb{gen}_{oc}_{e}",
                          tag="ocsb", bufs=2 * E)
            nc.scalar.dma_start(t[:], ins["ocsb"][e * 128:(e + 1) * 128,
                                                  oc * ow:(oc + 1) * ow])
            ocsb_t[(gen, oc, e)] = t

    # ---- per-half router pieces ----
    def router_mm(hf, h):
        if hf not in logits:
            logits[hf] = psum.tile([E, 512], F32, name=f"logits{hf}",
                                   tag="plog", bufs=1)
        nc.tensor.matmul(logits[hf][:], gwp[:, h * E:(h + 1) * E],
                         xh[(hf, h)][:], start=(h == 0), stop=(h == NH - 1))

    exs = {}
    rcps = {}

    def softmax_a(hf):
        ex = pool.tile([E, 512], BF16, name=f"ex{hf}", tag="ex", bufs=2)
        nc.scalar.activation(ex[:], logits.pop(hf)[:], AF.Exp)
        exs[hf] = ex
        ssum = psum.tile([1, 512], F32, name=f"ssum{hf}", tag="psm", bufs=3)
        nc.tensor.matmul(ssum[:], ones_e1[:], ex[:], start=True, stop=True)
        rcp32 = pool.tile([1, 512], F32, name=f"rcp32{hf}", tag="rcp32",
                          bufs=2)
        nc.vector.reciprocal_approx_fast(rcp32[:], ssum[:])
        rcpb = pool.tile([1, 512], BF16, name=f"rcpb{hf}", tag="rcpb", bufs=2)
        nc.vector.tensor_copy(rcpb[:], rcp32[:])
        rcps[hf] = rcpb

    def softmax_b(hf):
        bc = psum.tile([E, 512], F32, name=f"bc{hf}", tag="psm", bufs=3)
        nc.tensor.matmul(bc[:], ones_1e[:], rcps.pop(hf)[:], start=True,
                         stop=True)
        rt = pool.tile([E, 512], BF16, name=f"rt{hf}", tag="rt", bufs=2)
        nc.vector.tensor_tensor(rt[:], exs.pop(hf)[:], bc[:], OP.mult)
        rts[hf] = rt
        for j in range(4):
            rtp = psum.tile([128, E], BF16, name=f"rtp{hf}_{j}", tag="psm",
                            bufs=3)
            nc.tensor.transpose(rtp[:], rt[:, j * 128:(j + 1) * 128], eye_e[:])
            r = pool.tile([128, E], F32, name=f"rj{hf}_{j}",
                          tag=f"rj{hf}_{j}", bufs=1)
            nc.vector.tensor_copy(r[:], rtp[:])
            rj[hf * 4 + j] = r

    def in_scale(hf, h):
        # x_s = x * (r @ ics), written in place over the x tile
        isp = psum.tile([128, 512], F32, name=f"isp{hf}_{h}", tag="psm",
                        bufs=3)
        nc.tensor.matmul(isp[:], ics_t[:, h * 128:(h + 1) * 128],
                         rts[hf][:], start=True, stop=True)
        xt = xh[(hf, h)]
        if h % 3 != 2:
            nc.vector.tensor_tensor(xt[:], xt[:], isp[:], OP.mult)
        else:
            # gpsimd TT is ~2x slower than DVE; give it 1/3 of the tiles
            icp = pool.tile([128, 512], F32, name=f"icp{hf}_{h}", tag="icp",
                            bufs=3)
            nc.scalar.activation(icp[:], isp[:], AF.Copy)
            nc.gpsimd.tensor_tensor(xt[:], xt[:], icp[:], OP.mult)

    # ---- main-phase window pieces ----
    def emit_os(gen, oc, tcs):
        for tcc in tcs:
            t = pool.tile([128, ow], F32, name=f"os{gen}_{oc}_{tcc}",
                          tag="os", bufs=8)
            r = rj[tcc]
            nc.vector.tensor_scalar_mul(t[:], ocsb_t[(gen, oc, 0)][:],
                                        r[:, 0:1])
            for e in range(1, E):
                nc.vector.scalar_tensor_tensor(t[:], ocsb_t[(gen, oc, e)][:],
                                               r[:, e:e + 1], t[:],
                                               OP.mult, OP.add)
            os_t[(oc, tcc)] = t

    def emit_drain(gen, oc, tcc):
        yt = pool.tile([128, ow], F32, name=f"yt{gen}_{oc}_{tcc}", tag="yt",
                       bufs=6)
        nc.vector.tensor_tensor(yt[:], mains.pop((oc, tcc))[:],
                                os_t.pop((oc, tcc))[:], OP.mult)
        yt2 = pool.tile([128, ow], F32, name=f"yt2{gen}_{oc}_{tcc}",
                        tag="yt2", bufs=6)
        nc.gpsimd.tensor_tensor(yt2[:], yt[:], bias_t[(gen, oc)][:], OP.add)
        nc.scalar.dma_start(y[tcc * 128:(tcc + 1) * 128,
                              oc * ow:(oc + 1) * ow], yt2[:])

    # ---- prologue: half A router + softmax; in_scale A fused into w0 ----
    for h in range(NH):
        dma_x(0, h)
        dma_consts(h)
        router_mm(0, h)
    softmax_a(0)
    softmax_b(0)
    dma_oc_consts(0, 0)
    for h in range(NH):
        dma_w(0, 0, h)
    in_scale(0, 0)
    in_scale(0, 1)
    emit_os(0, 0, [0, 1])
    emit_os(0, 0, [2, 3])

    # ---- main: windows (gen, oc, pair); gen 0 = half A (tc 0-3),
    #      gen 1 = half B (tc 4-7). Window = 2 PSUM banks, ring of 4. ----
    windows = [(g, oc, p) for g in range(2) for oc in range(OC)
               for p in range(2)]

    # extra ops (half-B prologue) interleaved into half-A windows, per h step
    extras = {}                      # (widx, h) -> list of callables
    for h in range(NH):              # window 0: stream x half B,
        extras.setdefault((0, h), []).append(lambda h=h: dma_x(1, h))
    for h in range(NH - 2):          # ... and in_scale A two steps ahead
        extras.setdefault((0, h), []).append(lambda h=h: in_scale(0, h + 2))
    for h2 in range(NH):             # windows 1-2: router B (1 mm / step)
        w = 1 + h2 // 16
        extras.setdefault((w, (h2 * 2) % 32), []).append(
            lambda h2=h2: router_mm(1, h2))
    extras.setdefault((3, 0), []).append(lambda: softmax_a(1))
    extras.setdefault((3, 16), []).append(lambda: softmax_b(1))
    for h2 in range(NH):             # windows 4-14: in_scale B (3 / window)
        w = 4 + h2 // 3
        extras.setdefault((w, (h2 % 3) * 10 + 5), []).append(
            lambda h2=h2: in_scale(1, h2))

    prev = None
    for widx, (gen, oc, p) in enumerate(windows):
        tcs = [gen * 4 + p * 2, gen * 4 + p * 2 + 1]
        last = widx == len(windows) - 1
        if prev is not None:
            for tcc in prev[1]:
                emit_drain(prev[0], prev[2], tcc)
        if widx >= 2:
            emit_os(gen, oc, tcs)
        for tcc in tcs:
            mains[(oc, tcc)] = psum.tile([128, ow], F32,
                                         name=f"mp{gen}_{oc}_{tcc}",
                                         tag="mps", bufs=4)
        if p == 1:
            ngen, noc = (gen, oc + 1) if oc + 1 < OC else (gen + 1, 0)
            if ngen < 2:
                dma_oc_consts(ngen, noc)
        if last:
            # column-major: finish tc by tc so the tail drain is short
            for tcc in tcs:
                for h in range(NH):
                    w = wst[(gen, oc, h)] if tcc == tcs[0] else \
                        wst.pop((gen, oc, h))
                    nc.tensor.matmul(mains[(oc, tcc)][:],
                                     xh[(gen, h)][:, (tcc % 4) * 128:
                                                   (tcc % 4 + 1) * 128],
                                     w[:], start=(h == 0), stop=(h == NH - 1))
                if tcc != tcs[1]:
                    emit_drain(gen, oc, tcc)
            # final tc drained in halves so DVE/GpSimd/DMA pipeline
            tcc = tcs[1]
            mp = mains.pop((oc, tcc))
            ot = os_t.pop((oc, tcc))
            for k in range(2):
                sl = slice(k * (ow // 2), (k + 1) * (ow // 2))
                yt = pool.tile([128, ow // 2], F32, name=f"ytf{k}", tag="ytf",
                               bufs=2)
                nc.vector.tensor_tensor(yt[:], mp[:, sl], ot[:, sl], OP.mult)
                yt2 = pool.tile([128, ow // 2], F32, name=f"ytf2{k}",
                                tag="ytf2", bufs=2)
                nc.vector.tensor_tensor(yt2[:], yt[:],
                                        bias_t[(gen, oc)][:, sl], OP.add)
                nc.scalar.dma_start(y[tcc * 128:(tcc + 1) * 128,
                                      oc * ow + k * (ow // 2):
                                      oc * ow + (k + 1) * (ow // 2)], yt2[:])
            prev = None
            break
        for h in range(NH):
            for fn in extras.pop((widx, h), ()):
                fn()
            if p == 1:
                ngen, noc = (gen, oc + 1) if oc + 1 < OC else (gen + 1, 0)
                if ngen < 2:
                    dma_w(ngen, noc, h)
            w = wst[(gen, oc, h)] if p == 0 else wst.pop((gen, oc, h))
            for tcc in tcs:
                nc.tensor.matmul(mains[(oc, tcc)][:],
                                 xh[(gen, h)][:, (tcc % 4) * 128:
                                               (tcc % 4 + 1) * 128],
                                 w[:], start=(h == 0), stop=(h == NH - 1))
        prev = (gen, tcs, oc)
    assert not extras, f"unconsumed extras: {list(extras)[:4]}"


import numpy as np
import ml_dtypes

NCORES = 8
B, S, H, O, E = 4, 2048, 4096, 4096, 4
N = B * S
Nc = N // NCORES
CFG = dict(H=H, O=O, Nc=Nc, E=E, ow=512)
BF16_NP = ml_dtypes.bfloat16
FP8_NP = ml_dtypes.float8_e4m3

TRACE = False
LAST_EXEC_NS = None
LAST_TRACE_PATH = None
_NC_CACHE = None


def _get_nc():
    global _NC_CACHE
    if _NC_CACHE is None:
        import concourse.bacc as bacc
        import concourse.tile as tile
        nc = bacc.Bacc("TRN2", target_bir_lowering=False, debug=False,
                       num_devices=NCORES)
        ins_aps = {
            "xT": nc.dram_tensor("xT", [H, Nc], BF16, kind="ExternalInput").ap(),
            "wbT": nc.dram_tensor("wbT", [H, O], FP8, kind="ExternalInput").ap(),
            "gwP": nc.dram_tensor("gwP", [128, (H // 128) * E], BF16,
                                  kind="ExternalInput").ap(),
            "ics": nc.dram_tensor("ics", [E, H], BF16, kind="ExternalInput").ap(),
            "ocsb": nc.dram_tensor("ocsb", [E * 128, O], F32,
                                   kind="ExternalInput").ap(),
            "bias2": nc.dram_tensor("bias2", [128, O], F32,
                                    kind="ExternalInput").ap(),
            "ones_e": nc.dram_tensor("ones_e", [E, E], BF16,
                                     kind="ExternalInput").ap(),
            "eye_e": nc.dram_tensor("eye_e", [E, E], BF16,
                                    kind="ExternalInput").ap(),
        }
        outs_aps = {"y": nc.dram_tensor("y", [Nc, O], F32,
                                        kind="ExternalOutput").ap()}
        with tile.TileContext(nc) as tc:
            with ExitStack() as ctx:
                build_moe8(ctx, tc, outs_aps, ins_aps, CFG)
        nc.compile()
        _NC_CACHE = nc
    return _NC_CACHE


def kernel(x, weight, bias, gate_w, in_channel_scale, out_channel_scale):
    """Full inputs in, full output out; distributes over 8 NeuronCores."""
    global LAST_EXEC_NS, LAST_TRACE_PATH
    from concourse.bass_utils import run_bass_kernel_spmd

    x = np.asarray(x, dtype=np.float32)
    weight = np.asarray(weight, dtype=np.float32)
    bias = np.asarray(bias, dtype=np.float32)
    gate_w = np.asarray(gate_w, dtype=np.float32)
    ics = np.asarray(in_channel_scale, dtype=np.float32)
    ocs = np.asarray(out_channel_scale, dtype=np.float32)

    nc = _get_nc()
    xf = x.reshape(N, H)
    wbT = np.sign(weight).T.astype(FP8_NP)
    gwP = np.ascontiguousarray(
        gate_w.T.reshape(H // 128, 128, E).transpose(1, 0, 2)
        .reshape(128, (H // 128) * E)).astype(BF16_NP)
    ics_b = ics.astype(BF16_NP)
    bias2 = np.ascontiguousarray(np.broadcast_to(bias[None, :], (128, O)))
    ocsb = np.ascontiguousarray(
        np.broadcast_to(ocs[:, None, :], (E, 128, O)).reshape(E * 128, O))
    ones_e = np.ones((E, E), dtype=BF16_NP)
    eye_e = np.eye(E, dtype=BF16_NP)
    in_maps = []
    for c in range(NCORES):
        in_maps.append({
            "xT": np.ascontiguousarray(xf[c * Nc:(c + 1) * Nc, :].T).astype(BF16_NP),
            "wbT": wbT, "gwP": gwP, "ics": ics_b, "ocsb": ocsb,
            "bias2": bias2, "ones_e": ones_e, "eye_e": eye_e,
        })
    res = run_bass_kernel_spmd(nc, in_maps, core_ids=list(range(NCORES)),
                               trace=TRACE)
    if TRACE:
        LAST_EXEC_NS = res.exec_time_ns
        if res.instructions_and_trace:
            LAST_TRACE_PATH = res.instructions_and_trace[1]
    yfull = np.concatenate([res.results[c]["y"] for c in range(NCORES)], axis=0)
    return yfull.reshape(B, S, O)
